# revision 3
# baseline (speedup 1.0000x reference)
"""Trainium2 Bass kernel for nn_MetricLearningLoss (N=8192, D=128, C=100).

Math: with d2[i,j] = ||x_i - x_j||^2 and per-class (over ALL N rows)
n_c, M_c = sum of member rows, SS_c = sum of member squared norms:
  same_sum  = sum_c [ 2*n_c*SS_c - 2*||M_c||^2 ]
  loss = C_SS*SS_tot + C_MSQ*||M_tot||^2 + C_SAME*same_sum

Sharding: by FEATURE COLUMNS (D=128 -> 16 per core).  Every term above
decomposes over column slices (n_c depends only on labels, which every core
has in full), so each core computes a partial scalar loss for its 16-column
slice with NO cross-core communication, and the host's gather step sums the
8 partial scalars.  This removes the collective entirely (the cost model
charges a fixed 15us minimum per CollectiveCompute, which dominated the
44us baseline).

Inputs are fed to the device in bf16 (the 2e-2 harness tolerance dwarfs the
~1e-4 this costs): the PE runs at 1 cycle/row instead of 4, and the one-hot
build hits the DVE 2x 16-bit path.

Per-core plan (engines):
  scalar x tiles 0..31 DMA (own HWDGE ring; nothing queued ahead of it)
  sync   x tiles 32..63 DMA
  gpsimd labels DMA (SWDGE), iota, ones-column memset, one-hot tiles
         0..POOL_HT-1, class-axis (C) reduction of res, loss DMA
  vector iota/label copies, EARLY one-hot tiles (keeps DVE busy past the
         x-DMA completion so its xsa wait is evaluated against an
         already-set semaphore value instead of sleeping until the late
         completion notification), x^2 chunks written straight into the
         rhs tiles, remaining one-hot tiles, PSUM->SBUF copy + final chain
  tensor one accumulating chain of 64 bf16 matmuls
         px[100, 33] += H_t^T @ [x_t | 1 | x_t^2]   (t = 0..63)
         -> px = [M_c | n_c | SQ_c], SS_c = row-sum of SQ_c (one reduce)

Raw Bass (no TileContext), all cross-engine and same-engine dependencies
sequenced with explicit semaphores (the sim race detector requires explicit
waits even between dependent back-to-back ops on one engine).
"""

from contextlib import ExitStack

import ml_dtypes
import numpy as np

import concourse.bass as bass
import concourse.mybir as mybir
from concourse.bass_utils import run_bass_kernel_spmd

N, D, C = 8192, 128, 100
CORES = 8
DS = D // CORES          # 16 columns per core
KT = N // 128            # 64 row tiles of 128 rows
TW = 2 * DS + 1          # 33: [x (16) | one | sq (16)]
X_OFF, ONE_OFF, SQ_OFF = 0, DS, DS + 1
SIGMA, OMEGA = 0.2, 1.0
C_SAME = -(0.5 / (2 * SIGMA**2) + 0.5 / (2 * OMEGA**2))  # -6.5
C_SS = (0.5 / (2 * OMEGA**2)) * 2 * N                    # 4096
C_MSQ = -(0.5 / (2 * OMEGA**2)) * 2                      # -0.5
F32 = mybir.dt.float32
BF16 = mybir.dt.bfloat16
I32 = mybir.dt.int32

POOL_HT = 37             # one-hot tiles 0..POOL_HT-1 on gpsimd
EARLY_HT = 53            # DVE builds EARLY_HT..63 before the sq chain
SQ_CHUNKS = ((0, 8), (8, 16), (16, 32), (32, 48), (48, 64))
N_EARLY = KT - EARLY_HT
N_LATE = EARLY_HT - POOL_HT
HW = C + 1               # one-hot tile width: 100 classes + all-ones col

add = mybir.AluOpType.add
mult = mybir.AluOpType.mult
subtract = mybir.AluOpType.subtract
is_equal = mybir.AluOpType.is_equal
X = mybir.AxisListType.X
CAX = mybir.AxisListType.C

# res layout: [M 0..15 | n 16 | SQ 17..32 | SS 33 | fin 34..50]
# rows 0..99 (classes): fin = [2*C_SAME*n_c*SS_c | -2*C_SAME*M_c*M_c]
# row 100 (totals):     fin = [C_SS*SS_tot      | C_MSQ*M_tot*M_tot]
# so reduce(res[0:101, 34:51]) == loss.
RW = TW + 2 + DS
R_N, R_SS, R_NSS, R_M2 = DS, TW, TW + 1, TW + 2

# vsem numbering
V_EARLY0 = 6                       # first early one-hot
V_SQ0 = V_EARLY0 + N_EARLY         # first sq chunk (mult only)
V_LATE0 = V_SQ0 + len(SQ_CHUNKS)   # first late one-hot
V_COPY = V_LATE0 + N_LATE          # PSUM -> SBUF copy of px
V_RES = V_COPY + 3                 # res fully written


def build():
    nc = bass.Bass()
    x_in = nc.dram_tensor("x", [N, TW], BF16, kind="ExternalInput")
    lab_in = nc.dram_tensor("labels", [N], I32, kind="ExternalInput")
    loss_out = nc.dram_tensor("loss", [1], F32, kind="ExternalOutput")

    with ExitStack() as ctx:
        def sb(name, shape, dtype=F32):
            return ctx.enter_context(nc.sbuf_tensor(name, shape, dtype))

        iota_i = sb("iota_i", [128, C], I32)
        iota_f = sb("iota_f", [128, C], BF16)
        iota_p = sb("iota_p", [128, 1], I32)   # partition index column
        eq100 = sb("eq100", [128, 1])
        cf_nss = sb("cf_nss", [128, 1])        # 2*C_SAME, but 1/2 on row 100
        cf_m2 = sb("cf_m2", [128, 1])          # -2*C_SAME, but C_MSQ on row 100
        lab_i = sb("lab_i", [128, KT], I32)
        lab_f = sb("lab_f", [128, KT])
        # row r = p*64 + t lives at partition p, tile t
        x_all = sb("x_all", [128, KT * TW], BF16)
        hts = sb("hts", [128, KT * HW], BF16)
        res = sb("res", [128, RW])
        loss_sb = sb("loss_sb", [128, 1])

        px = ctx.enter_context(nc.psum_tensor([128, TW], F32))

        dsem = ctx.enter_context(nc.semaphore("dsem"))   # loss DMA
        lsem = ctx.enter_context(nc.semaphore("lsem"))   # labels DMA
        xsa = ctx.enter_context(nc.semaphore("xsa"))     # x tiles 0..31
        xsb = ctx.enter_context(nc.semaphore("xsb"))     # x tiles 32..63
        vsem = ctx.enter_context(nc.semaphore("vsem"))   # DVE progress
        gsem = ctx.enter_context(nc.semaphore("gsem"))   # Pool progress
        psem = ctx.enter_context(nc.semaphore("psem"))   # PE progress

        block = ctx.enter_context(nc.Block())

        xr = x_in.rearrange("(p t) d -> p (t d)", t=KT)  # [128, 64*33]
        xv = x_all[:].rearrange("p (t w) -> p t w", w=TW)

        def one_hot(eng, t):
            return eng.tensor_scalar(
                hts[:, t * HW:t * HW + C], iota_f[:], lab_f[:, t:t + 1],
                None, is_equal,
            )

        @block.scalar
        def _(sc):
            # fully contiguous halves: the host pre-pads each row to
            # [x (16) | 1 | zeros (16)] so no strided DMA is needed (strided
            # DMA destinations proved unreliable on real hardware).
            sc.dma_start(
                out=x_all[:, 0:32 * TW], in_=xr[:, 0:32 * TW],
            ).then_inc(xsa, 16)

        @block.sync
        def _(sync):
            sync.dma_start(
                out=x_all[:, 32 * TW:], in_=xr[:, 32 * TW:],
            ).then_inc(xsb, 16)

        @block.gpsimd
        def _(g):
            # iota first (unblocks the DVE iota copy), then labels on the
            # SWDGE ring: both HWDGE rings carry x, and the label completion
            # is what unblocks the whole one-hot front.
            g.iota(iota_i[:], pattern=[[1, C]], base=0, channel_multiplier=0
                   ).then_inc(gsem, 1)                   # 1
            g.dma_start(
                out=lab_i[:], in_=lab_in[:].rearrange("(p t) -> p t", t=KT)
            ).then_inc(lsem, 16)
            # iota_p AFTER the label DMA: its (fast) completion notification
            # wakes DVE right after the label sem value is set, so DVE's lsem
            # wait is evaluated against an already-set value.
            g.iota(iota_p[:], pattern=[[1, 1]], base=0, channel_multiplier=1
                   ).then_inc(gsem, 1)                   # 2
            g.memset(hts[:].rearrange("p (t w) -> p t w", w=HW)[:, :, C],
                     1.0).then_inc(gsem, 1)              # 3 (totals column)
            g.wait_ge(vsem, 5)                           # lab_f + iota_f done
            for t in range(POOL_HT):                     # gsem 4..
                one_hot(g, t).then_inc(gsem, 1)
            # tail: one full reduction of the prefolded strip IS the loss
            g.wait_ge(vsem, V_RES)
            g.tensor_reduce(out=loss_sb[0:1, 0:1], in_=res[0:C + 1, R_NSS:RW],
                            axis=mybir.AxisListType.XYZWC,
                            op=add).then_inc(gsem, 1)    # 4+POOL_HT
            g.wait_ge(gsem, 4 + POOL_HT)
            g.dma_start(out=loss_out[:], in_=loss_sb[0:1, 0:1]).then_inc(dsem, 16)

        @block.vector
        def _(v):
            v.wait_ge(gsem, 1)
            v.tensor_copy(iota_f[:], iota_i[:]).then_inc(vsem, 1)   # 1
            v.wait_ge(gsem, 2)
            v.tensor_scalar(eq100[:], iota_p[:], 100, None,
                            is_equal).then_inc(vsem, 1)             # 2
            v.wait_ge(vsem, 2)
            v.tensor_scalar(cf_m2[:], eq100[:], float(C_MSQ + 2 * C_SAME),
                            float(-2 * C_SAME), mult, add).then_inc(vsem, 1)  # 3
            v.tensor_scalar(cf_nss[:], eq100[:], float(0.5 - 2 * C_SAME),
                            float(2 * C_SAME), mult, add).then_inc(vsem, 1)   # 4
            v.wait_ge(lsem, 16)
            v.tensor_copy(lab_f[:], lab_i[:]).then_inc(vsem, 1)     # 5
            v.wait_ge(vsem, 5)
            for t in range(EARLY_HT, KT):
                one_hot(v, t).then_inc(vsem, 1)
            vc = V_SQ0 - 1
            # sq chunks: x^2 written straight into the rhs tiles.  The early
            # one-hots above kept DVE busy past the x DMA completion, so this
            # wait is evaluated against an already-set semaphore value.
            v.wait_ge(xsa, 16)
            for (t0, t1) in SQ_CHUNKS:
                if t0 == 32:
                    v.wait_ge(xsb, 16)
                v.tensor_tensor(
                    xv[:, t0:t1, SQ_OFF:SQ_OFF + DS],
                    xv[:, t0:t1, X_OFF:X_OFF + DS],
                    xv[:, t0:t1, X_OFF:X_OFF + DS],
                    mult,
                ).then_inc(vsem, 1)
                vc += 1
            for t in range(POOL_HT, EARLY_HT):
                one_hot(v, t).then_inc(vsem, 1)
                vc += 1
            assert vc == V_COPY - 1
            # ---- tail ----
            v.wait_ge(psem, KT)                          # px accumulated
            v.tensor_copy(res[0:C + 1, 0:TW], px[0:C + 1, :]).then_inc(vsem, 1)
            v.wait_ge(vsem, V_COPY)
            v.tensor_reduce(out=res[0:C + 1, R_SS:R_SS + 1],
                            in_=res[0:C + 1, SQ_OFF:SQ_OFF + DS],
                            axis=X, op=add).then_inc(vsem, 1)
            v.wait_ge(vsem, V_COPY + 1)
            # per-partition coefficient columns fold the class rows and the
            # totals row (row 100) into one uniform pair of ops; the 0.5 on
            # row 100 of cf_nss works because C_SS == N/2 and n[100] == N.
            v.scalar_tensor_tensor(
                res[0:C + 1, R_NSS:R_NSS + 1], res[0:C + 1, R_N:R_N + 1],
                cf_nss[0:C + 1, :], res[0:C + 1, R_SS:R_SS + 1],
                mult, mult).then_inc(vsem, 1)
            v.scalar_tensor_tensor(
                res[0:C + 1, R_M2:R_M2 + DS], res[0:C + 1, 0:DS],
                cf_m2[0:C + 1, :], res[0:C + 1, 0:DS],
                mult, mult).then_inc(vsem, 1)            # V_RES

        @block.tensor
        def _(te):
            sq_ready = {t0: V_SQ0 + i for i, (t0, t1) in enumerate(SQ_CHUNKS)}
            for t in range(KT):
                if t == 0:
                    te.wait_ge(gsem, 3)                  # totals column
                # x_all data deps flow transitively through the sq-chunk sems
                # (DVE waited xsa/xsb before squaring the same columns).
                if t in sq_ready:
                    te.wait_ge(vsem, sq_ready[t])
                if t < POOL_HT:
                    te.wait_ge(gsem, 4 + t)              # ht_t (Pool)
                elif t < EARLY_HT:
                    te.wait_ge(vsem, V_LATE0 + (t - POOL_HT))
                else:
                    te.wait_ge(vsem, V_EARLY0 + (t - EARLY_HT))
                te.matmul(px[0:C + 1, :], lhsT=hts[:, t * HW:(t + 1) * HW],
                          rhs=x_all[:, t * TW:(t + 1) * TW],
                          start=(t == 0), stop=(t == KT - 1)).then_inc(psem, 1)

    return nc


def make_in_maps(outputs, labels):
    x = np.asarray(outputs, dtype=np.float32)
    lab = np.ascontiguousarray(np.asarray(labels).astype(np.int32))
    assert x.shape == (N, D) and lab.shape == (N,)
    in_maps = []
    pad = np.zeros((N, TW - DS), dtype=ml_dtypes.bfloat16)
    pad[:, 0] = 1.0                      # the rhs "ones" column
    for m in range(CORES):
        xs = np.ascontiguousarray(np.concatenate(
            [x[:, m * DS:(m + 1) * DS].astype(ml_dtypes.bfloat16), pad],
            axis=1))
        in_maps.append({"x": xs, "labels": lab})
    return in_maps


def run(outputs, labels, **kwargs):
    nc = build()
    in_maps = make_in_maps(outputs, labels)
    return run_bass_kernel_spmd(nc, in_maps, core_ids=list(range(CORES)), **kwargs)


def kernel(outputs, labels):
    res = run(outputs, labels)
    total = 0.0
    for m in range(CORES):
        total += float(np.asarray(res.results[m]["loss"])[0])
    return np.float32(total).reshape(())


# revision 4
# speedup vs baseline: 1.0063x; 1.0063x over previous
"""Trainium2 Bass kernel for nn_MetricLearningLoss (N=8192, D=128, C=100).

Math: with d2[i,j] = ||x_i - x_j||^2 and per-class (over ALL N rows)
n_c, M_c = sum of member rows, SS_c = sum of member squared norms:
  same_sum  = sum_c [ 2*n_c*SS_c - 2*||M_c||^2 ]
  loss = C_SS*SS_tot + C_MSQ*||M_tot||^2 + C_SAME*same_sum

Sharding: by FEATURE COLUMNS (D=128 -> 16 per core).  Every term above
decomposes over column slices (n_c depends only on labels, which every core
has in full), so each core computes a partial scalar loss for its 16-column
slice with NO cross-core communication, and the host's gather step sums the
8 partial scalars.  This removes the collective entirely (the cost model
charges a fixed 15us minimum per CollectiveCompute, which dominated the
44us baseline).

Inputs are fed to the device in bf16 (the 2e-2 harness tolerance dwarfs the
~1e-4 this costs): the PE runs at 1 cycle/row instead of 4, and the one-hot
build hits the DVE 2x 16-bit path.

Per-core plan (engines):
  scalar x tiles 0..31 DMA (own HWDGE ring; nothing queued ahead of it)
  sync   x tiles 32..63 DMA
  gpsimd labels DMA (SWDGE), iota, ones-column memset, one-hot tiles
         0..POOL_HT-1, class-axis (C) reduction of res, loss DMA
  vector iota/label copies, EARLY one-hot tiles (keeps DVE busy past the
         x-DMA completion so its xsa wait is evaluated against an
         already-set semaphore value instead of sleeping until the late
         completion notification), x^2 chunks written straight into the
         rhs tiles, remaining one-hot tiles, PSUM->SBUF copy + final chain
  tensor one accumulating chain of 64 bf16 matmuls
         px[100, 33] += H_t^T @ [x_t | 1 | x_t^2]   (t = 0..63)
         -> px = [M_c | n_c | SQ_c], SS_c = row-sum of SQ_c (one reduce)

Raw Bass (no TileContext), all cross-engine and same-engine dependencies
sequenced with explicit semaphores (the sim race detector requires explicit
waits even between dependent back-to-back ops on one engine).
"""

from contextlib import ExitStack

import ml_dtypes
import numpy as np

import concourse.bass as bass
import concourse.mybir as mybir
from concourse.bass_utils import run_bass_kernel_spmd

N, D, C = 8192, 128, 100
CORES = 8
DS = D // CORES          # 16 columns per core
KT = N // 128            # 64 row tiles of 128 rows
TW = 2 * DS + 1          # 33: [x (16) | one | sq (16)]
X_OFF, ONE_OFF, SQ_OFF = 0, DS, DS + 1
SIGMA, OMEGA = 0.2, 1.0
C_SAME = -(0.5 / (2 * SIGMA**2) + 0.5 / (2 * OMEGA**2))  # -6.5
C_SS = (0.5 / (2 * OMEGA**2)) * 2 * N                    # 4096
C_MSQ = -(0.5 / (2 * OMEGA**2)) * 2                      # -0.5
F32 = mybir.dt.float32
BF16 = mybir.dt.bfloat16
I32 = mybir.dt.int32

POOL_HT = 36             # one-hot tiles 0..POOL_HT-1 on gpsimd
EARLY_HT = 52            # DVE builds EARLY_HT..63 before the sq chain
SQ_CHUNKS = ((0, 8), (8, 16), (16, 32), (32, 48), (48, 64))
N_EARLY = KT - EARLY_HT
N_LATE = EARLY_HT - POOL_HT
HW = C + 1               # one-hot tile width: 100 classes + all-ones col

add = mybir.AluOpType.add
mult = mybir.AluOpType.mult
subtract = mybir.AluOpType.subtract
is_equal = mybir.AluOpType.is_equal
X = mybir.AxisListType.X
CAX = mybir.AxisListType.C

# res layout: [M 0..15 | n 16 | SQ 17..32 | SS 33 | fin 34..50]
# rows 0..99 (classes): fin = [2*C_SAME*n_c*SS_c | -2*C_SAME*M_c*M_c]
# row 100 (totals):     fin = [C_SS*SS_tot      | C_MSQ*M_tot*M_tot]
# so reduce(res[0:101, 34:51]) == loss.
RW = TW + 2 + DS
R_N, R_SS, R_NSS, R_M2 = DS, TW, TW + 1, TW + 2

# vsem numbering
V_EARLY0 = 6                       # first early one-hot
V_SQ0 = V_EARLY0 + N_EARLY         # first sq chunk (mult only)
V_LATE0 = V_SQ0 + len(SQ_CHUNKS)   # first late one-hot
V_COPY = V_LATE0 + N_LATE          # PSUM -> SBUF copy of px
V_RES = V_COPY + 3                 # res fully written


def build():
    nc = bass.Bass()
    x_in = nc.dram_tensor("x", [N, TW], BF16, kind="ExternalInput")
    lab_in = nc.dram_tensor("labels", [N], I32, kind="ExternalInput")
    loss_out = nc.dram_tensor("loss", [1], F32, kind="ExternalOutput")

    with ExitStack() as ctx:
        def sb(name, shape, dtype=F32):
            return ctx.enter_context(nc.sbuf_tensor(name, shape, dtype))

        iota_i = sb("iota_i", [128, C], I32)
        iota_f = sb("iota_f", [128, C], BF16)
        iota_p = sb("iota_p", [128, 1], I32)   # partition index column
        eq100 = sb("eq100", [128, 1])
        cf_nss = sb("cf_nss", [128, 1])        # 2*C_SAME, but 1/2 on row 100
        cf_m2 = sb("cf_m2", [128, 1])          # -2*C_SAME, but C_MSQ on row 100
        lab_i = sb("lab_i", [128, KT], I32)
        lab_f = sb("lab_f", [128, KT])
        # row r = p*64 + t lives at partition p, tile t
        x_all = sb("x_all", [128, KT * TW], BF16)
        hts = sb("hts", [128, KT * HW], BF16)
        res = sb("res", [128, RW])
        loss_sb = sb("loss_sb", [128, 1])

        px = ctx.enter_context(nc.psum_tensor([128, TW], F32))

        dsem = ctx.enter_context(nc.semaphore("dsem"))   # loss DMA
        lsem = ctx.enter_context(nc.semaphore("lsem"))   # labels DMA
        xsa = ctx.enter_context(nc.semaphore("xsa"))     # x tiles 0..31
        xsb = ctx.enter_context(nc.semaphore("xsb"))     # x tiles 32..63
        vsem = ctx.enter_context(nc.semaphore("vsem"))   # DVE progress
        gsem = ctx.enter_context(nc.semaphore("gsem"))   # Pool progress
        psem = ctx.enter_context(nc.semaphore("psem"))   # PE progress

        block = ctx.enter_context(nc.Block())

        xr = x_in.rearrange("(p t) d -> p (t d)", t=KT)  # [128, 64*33]
        xv = x_all[:].rearrange("p (t w) -> p t w", w=TW)

        def one_hot(eng, t):
            return eng.tensor_scalar(
                hts[:, t * HW:t * HW + C], iota_f[:], lab_f[:, t:t + 1],
                None, is_equal,
            )

        @block.scalar
        def _(sc):
            # fully contiguous halves: the host pre-pads each row to
            # [x (16) | 1 | zeros (16)] so no strided DMA is needed (strided
            # DMA destinations proved unreliable on real hardware).
            sc.dma_start(
                out=x_all[:, 0:32 * TW], in_=xr[:, 0:32 * TW],
            ).then_inc(xsa, 16)

        @block.sync
        def _(sync):
            sync.dma_start(
                out=x_all[:, 32 * TW:], in_=xr[:, 32 * TW:],
            ).then_inc(xsb, 16)

        @block.gpsimd
        def _(g):
            # iota first (unblocks the DVE iota copy), then labels on the
            # SWDGE ring: both HWDGE rings carry x, and the label completion
            # is what unblocks the whole one-hot front.
            g.iota(iota_i[:], pattern=[[1, C]], base=0, channel_multiplier=0
                   ).then_inc(gsem, 1)                   # 1
            g.dma_start(
                out=lab_i[:], in_=lab_in[:].rearrange("(p t) -> p t", t=KT)
            ).then_inc(lsem, 16)
            # iota_p AFTER the label DMA: its (fast) completion notification
            # wakes DVE right after the label sem value is set, so DVE's lsem
            # wait is evaluated against an already-set value.
            g.iota(iota_p[:], pattern=[[1, 1]], base=0, channel_multiplier=1
                   ).then_inc(gsem, 1)                   # 2
            g.memset(hts[:].rearrange("p (t w) -> p t w", w=HW)[:, :, C],
                     1.0).then_inc(gsem, 1)              # 3 (totals column)
            g.wait_ge(vsem, 5)                           # lab_f + iota_f done
            for t in range(POOL_HT):                     # gsem 4..
                one_hot(g, t).then_inc(gsem, 1)
            # tail: one full reduction of the prefolded strip IS the loss
            g.wait_ge(vsem, V_RES)
            g.tensor_reduce(out=loss_sb[0:1, 0:1], in_=res[0:C + 1, R_NSS:RW],
                            axis=mybir.AxisListType.XYZWC,
                            op=add).then_inc(gsem, 1)    # 4+POOL_HT
            g.wait_ge(gsem, 4 + POOL_HT)
            g.dma_start(out=loss_out[:], in_=loss_sb[0:1, 0:1]).then_inc(dsem, 16)

        @block.vector
        def _(v):
            v.wait_ge(gsem, 1)
            v.tensor_copy(iota_f[:], iota_i[:]).then_inc(vsem, 1)   # 1
            v.wait_ge(gsem, 2)
            v.tensor_scalar(eq100[:], iota_p[:], 100, None,
                            is_equal).then_inc(vsem, 1)             # 2
            v.wait_ge(vsem, 2)
            v.tensor_scalar(cf_m2[:], eq100[:], float(C_MSQ + 2 * C_SAME),
                            float(-2 * C_SAME), mult, add).then_inc(vsem, 1)  # 3
            v.tensor_scalar(cf_nss[:], eq100[:], float(0.5 - 2 * C_SAME),
                            float(2 * C_SAME), mult, add).then_inc(vsem, 1)   # 4
            v.wait_ge(lsem, 16)
            v.tensor_copy(lab_f[:], lab_i[:]).then_inc(vsem, 1)     # 5
            v.wait_ge(vsem, 5)
            for t in range(EARLY_HT, KT):
                one_hot(v, t).then_inc(vsem, 1)
            vc = V_SQ0 - 1
            # sq chunks: x^2 written straight into the rhs tiles.  The early
            # one-hots above kept DVE busy past the x DMA completion, so this
            # wait is evaluated against an already-set semaphore value.
            v.wait_ge(xsa, 16)
            for (t0, t1) in SQ_CHUNKS:
                if t0 == 32:
                    v.wait_ge(xsb, 16)
                v.tensor_tensor(
                    xv[:, t0:t1, SQ_OFF:SQ_OFF + DS],
                    xv[:, t0:t1, X_OFF:X_OFF + DS],
                    xv[:, t0:t1, X_OFF:X_OFF + DS],
                    mult,
                ).then_inc(vsem, 1)
                vc += 1
            for t in range(POOL_HT, EARLY_HT):
                one_hot(v, t).then_inc(vsem, 1)
                vc += 1
            assert vc == V_COPY - 1
            # ---- tail ----
            v.wait_ge(psem, KT)                          # px accumulated
            v.tensor_copy(res[0:C + 1, 0:TW], px[0:C + 1, :]).then_inc(vsem, 1)
            v.wait_ge(vsem, V_COPY)
            v.tensor_reduce(out=res[0:C + 1, R_SS:R_SS + 1],
                            in_=res[0:C + 1, SQ_OFF:SQ_OFF + DS],
                            axis=X, op=add).then_inc(vsem, 1)
            v.wait_ge(vsem, V_COPY + 1)
            # per-partition coefficient columns fold the class rows and the
            # totals row (row 100) into one uniform pair of ops; the 0.5 on
            # row 100 of cf_nss works because C_SS == N/2 and n[100] == N.
            v.scalar_tensor_tensor(
                res[0:C + 1, R_NSS:R_NSS + 1], res[0:C + 1, R_N:R_N + 1],
                cf_nss[0:C + 1, :], res[0:C + 1, R_SS:R_SS + 1],
                mult, mult).then_inc(vsem, 1)
            v.scalar_tensor_tensor(
                res[0:C + 1, R_M2:R_M2 + DS], res[0:C + 1, 0:DS],
                cf_m2[0:C + 1, :], res[0:C + 1, 0:DS],
                mult, mult).then_inc(vsem, 1)            # V_RES

        @block.tensor
        def _(te):
            sq_ready = {t0: V_SQ0 + i for i, (t0, t1) in enumerate(SQ_CHUNKS)}
            for t in range(KT):
                if t == 0:
                    te.wait_ge(gsem, 3)                  # totals column
                # x_all data deps flow transitively through the sq-chunk sems
                # (DVE waited xsa/xsb before squaring the same columns).
                if t in sq_ready:
                    te.wait_ge(vsem, sq_ready[t])
                if t < POOL_HT:
                    te.wait_ge(gsem, 4 + t)              # ht_t (Pool)
                elif t < EARLY_HT:
                    te.wait_ge(vsem, V_LATE0 + (t - POOL_HT))
                else:
                    te.wait_ge(vsem, V_EARLY0 + (t - EARLY_HT))
                te.matmul(px[0:C + 1, :], lhsT=hts[:, t * HW:(t + 1) * HW],
                          rhs=x_all[:, t * TW:(t + 1) * TW],
                          start=(t == 0), stop=(t == KT - 1)).then_inc(psem, 1)

    return nc


def make_in_maps(outputs, labels):
    x = np.asarray(outputs, dtype=np.float32)
    lab = np.ascontiguousarray(np.asarray(labels).astype(np.int32))
    assert x.shape == (N, D) and lab.shape == (N,)
    in_maps = []
    pad = np.zeros((N, TW - DS), dtype=ml_dtypes.bfloat16)
    pad[:, 0] = 1.0                      # the rhs "ones" column
    for m in range(CORES):
        xs = np.ascontiguousarray(np.concatenate(
            [x[:, m * DS:(m + 1) * DS].astype(ml_dtypes.bfloat16), pad],
            axis=1))
        in_maps.append({"x": xs, "labels": lab})
    return in_maps


def run(outputs, labels, **kwargs):
    nc = build()
    in_maps = make_in_maps(outputs, labels)
    return run_bass_kernel_spmd(nc, in_maps, core_ids=list(range(CORES)), **kwargs)


def kernel(outputs, labels):
    res = run(outputs, labels)
    total = 0.0
    for m in range(CORES):
        total += float(np.asarray(res.results[m]["loss"])[0])
    return np.float32(total).reshape(())
